# revision 26
# baseline (speedup 1.0000x reference)
"""KANLayer kernel for 8 Trainium2 NeuronCores (raw Bass, fp8 DoubleRow).

Reference computation (B=4096, D=1024, O=1024, S=4 spline points):
    xmin/xmax = per-feature min/max of x over the batch dim      # [1, D]
    xn  = (x - xmin) / (xmax - xmin)                             # [B, D]
    c   = spline_coeffs.sum(axis=2)                              # [O, D, 4]
    out = xn^3 @ c0.T + xn^2 @ c1.T + xn @ c2.T + c3.sum(d)     # [B, O]

Sharding: tensor-parallel over the output dim O. Core r owns output
columns [128r, 128r+128). Every core loads the full x (transposed to
[D, B] fp16 on the host) and computes all per-feature stats locally.

This environment executes NEFF engine instructions with a large fixed
per-instruction cost (~39us per PE matmul regardless of moving width
(re-measured this session: 192mm@512 == 192mm@256 per-iter time), DVE
~23us fp16-in / +44us per fp8 INPUT operand / ~62us reduce / ~32us
psum drain, waits ~11us) and engine instruction processing is mostly
serialized chip-side (DVE+PE concurrency probe: ~2/3 serialized), so
TOTAL instruction count dominates everything.

The fp16 floor is 192 matmuls (contraction 3*1024 / 128 partitions x
8 batch quarters; 512 moving rows is a hard walrus/ISA limit,
re-verified: ow=1024 fails birverifier). This kernel halves that to
**96 matmuls** via fp8e4m3 DoubleRow perf mode (256-row contraction
per instruction at the same ~39us fixed cost, verified on-device).

Pure-fp8 quantization of xn powers + coeffs fails the 2e-2 gate
(2.7e-2 measured on the real data). Fix: **centered basis**. fp8 error
is relative, so quantize the small-magnitude centered variable
    p1 = 2*xn - 1 = (2x - (mn+mx)) / (mx - mn)   in [-1, 1]
and its powers p2 = p1^2, p3 = p1^3 instead of xn^k, folding the
affine shift into the coefficients exactly on the host (weight
marshaling, f64):
    out = sum_d b3*p3 + b2*p2 + b1*p1 + bias[o],
    b3 = c0/8, b2 = (c1 + 1.5 c0)/4, b1 = (c2 + c1 + 0.75 c0)/2,
    bias[o] = sum_d c3 + c2/2 + c1/4 + c0/8.
Centered moving values have ~3.7x smaller magnitude (std 0.29 vs rms
0.55 of xn), shrinking absolute quantization error of BOTH factors.
Measured end-to-end (exact device-chain numpy model): max rel err
1.05e-2 vs the f32 reference (gate 2e-2); on-device chain verified
bit-exact vs that model (mb.py pipecheck).

Per-core instruction budget (116/iter):
  SP  : xt DMA + out DMA + 2 waits
  Pool: ONE coeff+bias DMA + 1 wait (the f32 bias bits are packed
        into the tail of the fp8 coefficient row on the host; an
        alloc_sbuf_tensor_at f32 alias over the arena tail reads them
        back -- DMA same-dtype copies are byte-exact)
  DVE : min,max reduces; s=mn+mx, rng=mx-mn, sc=1/rng (tiny);
        xt = 2*xt - s in place (scalar_tensor_tensor); xt = xt*sc in
        place (xt_all now holds p1 in f16 -- fp8-INPUT ops cost +44us
        per fp8 operand here, so all three fp8 tiles derive from the
        f16 p1: p2=p1*p1 and copy(p1) are pure-f16-in, p3=p2_8*p1
        pays for one fp8 operand); drain psum+bias -> fp16 stage;
        ~5 waits. The stats/normalize block of iteration it+1 is
        SOFTWARE-PIPELINED to sit between p3(it) and drain(it) so it
        can overlap the PE block (it touches only xt_all/stats).
  PE  : 96 DoubleRow matmuls (3 planes x 4 chunk-pairs x 8 batch
        quarters; accumulation group of 12 per PSUM bank) + 1 merged
        wait (s_pw carries both the coeff DMA +16 and DVE's p3 +1)

DoubleRow semantics (verified on-device): out = sum_i lhsT[:,i,:].T @
rhs[:,i,:] for i in {0,1}; rhs [128, 2, 512] may stride the pair dim
across d-chunk planes (stride B), lhsT [128, 2, 128] packs coefficient
pairs contiguously; contraction pair i covers d = (2t+i)*128 + p.

Same-engine RAW hazards on DVE are NOT checked by this runtime
(~2-deep lookahead), so the tiny stats chain carries s_dv waits;
equal-size streaming ops rely on the trailing-pointer property.

DRAM rows are padded (+64 elements) so DMA descriptors do not coalesce
into the slow single-engine contiguous path.

n_iters > 1 builds a NEFF that runs the whole kernel N times
back-to-back (device-time measurement by wall-clock slope; the axon
tunnel's per-call input shipping makes single-run wall time
meaningless). timing_mode keeps all big tensors internal.

Output per core is out_t [128, B] fp16 (transposed); the host
concatenates the 8 shards, crops the pad, transposes back, upcasts.
"""

import numpy as np

import concourse.bass as bass
import concourse.mybir as mybir
from concourse.bass_utils import run_bass_kernel_spmd

P = 128            # SBUF partitions / rows per tile
B = 4096           # batch
BP = B + 64        # padded DRAM row length (defeats descriptor coalescing)
D = 1024           # input features
O = 1024           # output features
S = 4              # spline points
NCORES = 8
OS = O // NCORES   # output columns per core = 128
DC = D // P        # d-chunks = 8
NT = DC // 2       # chunk pairs = 4
QW = 512           # matmul moving-dim width (one PSUM bank)
NQ = B // QW       # 8
NK = 3             # coefficient planes (for p3, p2, p1)
CF8 = NK * NT * 2 * OS   # packed fp8 coeffs per partition = 3072
CF8P = CF8 + 64          # padded row

F32 = mybir.dt.float32
F16 = mybir.dt.float16
F8 = mybir.dt.float8e4
AX = mybir.AxisListType
ALU = mybir.AluOpType
DR = mybir.MatmulPerfMode.DoubleRow

_CACHE = {}


def _bcast(ap2d, n, inner):
    """[P, n] tensor -> [P, n, inner] with stride-0 inner dim."""
    return bass.AP(
        tensor=ap2d.tensor,
        offset=ap2d.offset,
        ap=[[ap2d.ap[0][0], P], [1, n], [0, inner]],
    )


def _build_bass(n_iters: int = 1, timing_mode: bool = False) -> bass.Bass:
    nc = bass.Bass(num_devices=NCORES, dynamic_dma_scratch_size=8192)

    kind = {} if timing_mode else {"kind": "ExternalInput"}
    okind = {} if timing_mode else {"kind": "ExternalOutput"}
    xt = nc.dram_tensor("xt", [D, BP], F16, **kind)
    cf = nc.dram_tensor("cf", [P, CF8P], F8, **kind)
    out_t = nc.dram_tensor("out_t", [OS, BP], F16, **okind)
    dummy = (
        nc.dram_tensor("tout", [P, 2], F32, kind="ExternalOutput")
        if timing_mode
        else None
    )

    from contextlib import ExitStack

    ctx = ExitStack()
    with ctx:
        sem = lambda name: ctx.enter_context(nc.semaphore(name))  # noqa: E731
        s_ldx = sem("s_ldx")  # +16/iter: xt load done
        s_pw = sem("s_pw")    # +17/iter: power tiles ready (+1, DVE p3)
                              # AND coeff+bias load done (+16, pool DMA)
        s_mm = sem("s_mm")    # +17/iter: PE consumed everything (+1)
                              # AND output store done (+16, SP DMA)
        s_dr = sem("s_dr")    # +1/iter: PSUM drained to stage
        s_dv = sem("s_dv")    # +3/iter: DVE stats RAW chain
        s_fin = sem("s_fin")  # timing-mode init/final bookkeeping

        sb = lambda name, shape, dtype: ctx.enter_context(  # noqa: E731
            nc.sbuf_tensor(name, shape, dtype)
        )
        xt_all = sb("xt_all", [P, DC, B], F16)   # 64KB; becomes 2x-s in place
        p1t = sb("p1t", [P, DC, B], F8)          # 32KB
        p2t = sb("p2t", [P, DC, B], F8)          # 32KB
        p3t = sb("p3t", [P, DC, B], F8)          # 32KB
        # coeff arena pinned at the top of SBUF; an f32 alias over its
        # tail lets ONE fp8 DMA deliver both the coefficients and the
        # f32 bias bits (host packs the raw f32 bytes into the fp8 row)
        ARENA_OFF = 226144
        cfa = nc.alloc_sbuf_tensor_at(
            "cfa", [P, CF8 + 64], F8, offset=ARENA_OFF
        )
        bias = nc.alloc_sbuf_tensor_at(
            "bias32", [OS, 1], F32, offset=ARENA_OFF + CF8
        )

        def cf_ap(k, t):
            b = cfa[:, 0:1]
            return bass.AP(
                tensor=b.tensor,
                offset=(k * NT + t) * 2 * OS,
                ap=[[b.ap[0][0], P], [OS, 2], [1, OS]],
            )
        stage = sb("stage", [OS, B], F16)        # 8KB
        stats = sb("stats", [P, 5 * DC], F32)
        mn = stats[:, 0:DC]
        mx = stats[:, DC : 2 * DC]
        ssum = stats[:, 2 * DC : 3 * DC]   # mn+mx
        rng = stats[:, 3 * DC : 4 * DC]    # mx-mn
        sc = stats[:, 4 * DC : 5 * DC]     # 1/(mx-mn)

        psum = ctx.enter_context(nc.psum_tensor("ps", [P, B], F32))

        NI = n_iters

        with nc.Block() as block:

            @block.sync
            def _(sp):
                if timing_mode:
                    # one-time finite xt init (per-chunk rng = 0.5 so the
                    # reciprocal stays finite; cf/bs DRAM read as zeros,
                    # which is harmless)
                    sp.wait_ge(s_fin, 2)
                    z = xt_all[:, 0, :]
                    sp.dma_start(
                        out=xt[:, :].rearrange("(n p) f -> p n f", p=P),
                        in_=bass.AP(
                            tensor=z.tensor,
                            offset=z.offset,
                            ap=[[z.ap[0][0], P], [0, D // P], [1, BP]],
                        ),
                    ).then_inc(s_fin, 16)
                    sp.wait_ge(s_fin, 18)
                # loads are issued BEFORE the previous store so the
                # pipelined DVE (which needs xt(it+1) for the stats that
                # run under PE(it)) never waits on a store->drain cycle
                def load(it):
                    if it > 0:
                        # >= 17*it covers p3(it-1) (last xt_all read); the
                        # extra pool-DMA(it-1) condition is acyclic and
                        # completes in the same window
                        sp.wait_ge(s_pw, 17 * it)
                    sp.dma_start(
                        out=xt_all[:, :, :],
                        in_=xt[:, 0:B].rearrange("(j p) f -> p j f", p=P),
                    ).then_inc(s_ldx, 16)

                load(0)
                for it in range(1, NI):
                    load(it)
                    sp.wait_ge(s_dr, it)
                    sp.dma_start(
                        out=out_t[:, 0:B], in_=stage[:, :]
                    ).then_inc(s_mm, 16)
                sp.wait_ge(s_dr, NI)
                sp.dma_start(
                    out=out_t[:, 0:B], in_=stage[:, :]
                ).then_inc(s_mm, 16)
                sp.wait_ge(s_mm, 17 * NI)
                if dummy is not None:
                    sp.dma_start(out=dummy[:, :], in_=stats[:, 16:18]).then_inc(
                        s_fin, 16
                    )
                    sp.wait_ge(s_fin, 34)

            @block.gpsimd
            def _(pool):
                for it in range(NI):
                    if it > 0:
                        # s_dr(it) implies mm(it-1) done (cfs free) and the
                        # previous drain read bias
                        pool.wait_ge(s_dr, it)
                    pool.dma_start(
                        out=cfa[:, 0 : CF8 + 4], in_=cf[:, 0 : CF8 + 4]
                    ).then_inc(s_pw, 16)

            @block.vector
            def _(dve):
                if timing_mode:
                    dve.memset(xt_all[:, :, 0 : B // 2], 0.25).then_inc(s_fin)
                    dve.memset(xt_all[:, :, B // 2 : B], 0.75).then_inc(s_fin)

                def stats_norm(it):
                    """Reduce stats over xt(it) and turn xt_all into
                    p1 = 2*xn-1 in place (f16). Touches only xt_all and
                    the stats tile, so it runs UNDER the PE block of the
                    previous iteration (software pipeline)."""
                    V = 3 * it
                    dve.wait_ge(s_ldx, 16 * (it + 1))
                    dve.tensor_reduce(
                        mn, xt_all[:, :, :], axis=AX.X, op=ALU.min
                    ).then_inc(s_dv)
                    dve.tensor_reduce(
                        mx, xt_all[:, :, :], axis=AX.X, op=ALU.max
                    ).then_inc(s_dv)
                    dve.wait_ge(s_dv, V + 2)
                    dve.tensor_add(ssum, mn, mx)
                    dve.tensor_sub(rng, mx, mn).then_inc(s_dv)
                    dve.wait_ge(s_dv, V + 3)
                    dve.reciprocal(sc, rng)
                    # xt_all <- 2*xt_all - (mn+mx), in place; ssum read is
                    # 2 ops behind its write with a wait+recip between
                    dve.scalar_tensor_tensor(
                        xt_all[:, :, :],
                        xt_all[:, :, :],
                        2.0,
                        _bcast(ssum, DC, B),
                        ALU.mult,
                        ALU.subtract,
                    )
                    # xt_all <- p1 = (2x - s) * sc in [-1, 1], f16 in place
                    # (fp8-INPUT DVE ops cost ~+44us per fp8 operand in
                    # this environment, so keep p1 in fp16 and derive all
                    # three fp8 tiles from it: p2/copy are pure-f16-in
                    # ~23us, p3 pays for one fp8 operand ~70us)
                    dve.tensor_mul(
                        xt_all[:, :, :], xt_all[:, :, :], _bcast(sc, DC, B)
                    )

                def drain(it):
                    # psum(it) + bias -> fp16 stage (s_mm >= it+1 already
                    # awaited by the caller)
                    dve.tensor_scalar_add(
                        stage[:, :], psum[:, :], bias[:, 0:1]
                    ).then_inc(s_dr)

                stats_norm(0)
                for it in range(NI):
                    if it > 0:
                        # 17*it-16 <=> PE(it-1) done (+it) AND store(it-2)
                        # done (+16*(it-1)): psum full, p-tiles free, stage
                        # free. PE(it) and store(it-1) are blocked behind
                        # this point so the sum cannot be reached wrongly.
                        dve.wait_ge(s_mm, 17 * it - 16)
                        drain(it - 1)
                    with nc.allow_low_precision(reason="fp8 moving tiles"):
                        dve.tensor_mul(
                            p2t[:, :, :], xt_all[:, :, :], xt_all[:, :, :]
                        )
                        dve.tensor_copy(p1t[:, :, :], xt_all[:, :, :])
                        # last xt_all read; s_pw also hands xt_all back
                        # to SP for the next load
                        dve.tensor_mul(
                            p3t[:, :, :], p2t[:, :, :], xt_all[:, :, :]
                        ).then_inc(s_pw)
                    if it < NI - 1:
                        stats_norm(it + 1)  # overlaps PE(it)
                dve.wait_ge(s_mm, 17 * NI - 16)
                drain(NI - 1)

            @block.tensor
            def _(pe):
                for it in range(NI):
                    # single merged wait: 17*(it+1) = both the coeff+bias
                    # DMA (+16) and DVE's p3 (+1) of this iteration done;
                    # p3 also implies drain(it-1) (earlier on the DVE
                    # stream), so PSUM is consistent for start=True
                    pe.wait_ge(s_pw, 17 * (it + 1))
                    mm = None
                    for t in range(NT):
                        for k in range(NK):
                            src = [p3t, p2t, p1t][k]
                            for q in range(NQ):
                                mm = pe.matmul(
                                    psum[:, q * QW : (q + 1) * QW],
                                    lhsT=cf_ap(k, t),
                                    rhs=src[:, 2 * t : 2 * t + 2, q * QW : (q + 1) * QW],
                                    start=(t == 0 and k == 0),
                                    stop=(t == NT - 1 and k == NK - 1),
                                    perf_mode=DR,
                                )
                    mm.then_inc(s_mm)

    return nc


def get_bass(n_iters: int = 1, timing_mode: bool = False) -> bass.Bass:
    key = f"nc{n_iters}_{timing_mode}"
    if key not in _CACHE:
        _CACHE[key] = _build_bass(n_iters, timing_mode)
    return _CACHE[key]


def make_in_maps(x: np.ndarray, spline_coeffs: np.ndarray):
    """Host-side sharding/marshaling only (slicing, transposes, dtype casts,
    and the exact affine recombination of the spline weights)."""
    import ml_dtypes

    x = np.asarray(x, dtype=np.float32)
    c = np.asarray(spline_coeffs, dtype=np.float64).sum(axis=2)  # [O, D, 4]

    xtn = np.zeros((D, BP), dtype=np.float16)
    xtn[:, 0:B] = x.T.astype(np.float16)

    c0, c1, c2, c3 = c[..., 0], c[..., 1], c[..., 2], c[..., 3]
    b3 = c0 / 8.0
    b2 = (c1 + 1.5 * c0) / 4.0
    b1 = (c2 + c1 + 0.75 * c0) / 2.0
    bias_full = (c3 + 0.5 * c2 + 0.25 * c1 + 0.125 * c0).sum(axis=1)  # [O]

    f8 = ml_dtypes.float8_e4m3

    def pack(b):  # b [OS, D] -> [P][NT, 2, OS]
        t = b.T.reshape(NT, 2, P, OS)
        return t.transpose(2, 0, 1, 3)

    in_maps = []
    for r in range(NCORES):
        sl = slice(r * OS, (r + 1) * OS)
        planes = np.stack(
            [pack(b3[sl]), pack(b2[sl]), pack(b1[sl])], axis=1
        )  # [P, NK, NT, 2, OS]
        cf_np = np.zeros((P, CF8P), dtype=f8)
        cf_np[:, 0:CF8] = np.clip(
            planes.reshape(P, CF8), -240.0, 240.0
        ).astype(f8)
        cf_u8 = cf_np.view(np.uint8)
        cf_u8[:, CF8 : CF8 + 4] = (
            bias_full[sl].astype("<f4").view(np.uint8).reshape(OS, 4)
        )
        in_maps.append({"xt": xtn, "cf": cf_np})
    return in_maps


def assemble_output(results) -> np.ndarray:
    out = np.concatenate(
        [results[r]["out_t"][:, 0:B] for r in range(NCORES)], axis=0
    )
    return np.ascontiguousarray(out.T).astype(np.float32)  # [B, O]


def run(x: np.ndarray, spline_coeffs: np.ndarray, trace: bool = False,
        n_iters: int = 1):
    """Returns (output, BassKernelResults)."""
    nc = get_bass(n_iters)
    in_maps = make_in_maps(x, spline_coeffs)
    res = run_bass_kernel_spmd(nc, in_maps, list(range(NCORES)), trace=trace)
    return assemble_output(res.results), res


def kernel(x: np.ndarray, spline_coeffs: np.ndarray) -> np.ndarray:
    out, _ = run(x, spline_coeffs, trace=False)
    return out
